# revision 1
# baseline (speedup 1.0000x reference)
"""Trainium2 Bass kernel for BitNet multi-head attention (nn_MultiHeadAttention_62294205661880).

Sharding: 8 cores = 2 batches x 4 head-groups (4 heads each).  Each core
computes qkv projection, RoPE, causal attention and a column-parallel slice
of the output projection for its (batch, head-group); the host sums the 4
partial out-projections per batch (the tensor-parallel all-reduce done
host-side, since the contract gathers to host anyway).

BitNet quantization is folded on the host: weights are uploaded as exact
ternary {-1,0,+1} bf16 matrices; scale_qkv^2/sqrt(dh) is folded into the
softmax exp() scale and scale_qkv*scale_out into a final host-side scalar.

Device layout trick: everything is computed transposed.  Q_T/K_T come out of
the projection as [dh, S]; scores are computed as s_T[k, q]; the softmax
denominator sums over the partition (key) dim via an all-ones stationary
matmul (which also replicates the sums across partitions for free); AV
produces out_T[dh, q] which feeds the output projection directly.  No
on-device transposes at all.  Softmax skips the max-subtraction: scores are
bounded (~+-2) because the BitNet weight scale is tiny, so exp() is safe.
"""

import sys
import types

import numpy as np
import ml_dtypes

import concourse.bass as bass
import concourse.mybir as mybir
import concourse.tile as tile
from concourse import bacc
from concourse.bass_utils import run_bass_kernel_spmd

D_MODEL = 2048
N_HEADS = 16
D_HEAD = 128
SEQ = 2048
BATCH = 2
ROPE_BASE = 10000.0

N_CORES = 8
HPC = 4  # heads per core
R_LOCAL = HPC * D_HEAD  # 512 local q (or k, or v) rows per core
MO = D_MODEL // 128  # 16 contraction blocks
NKI = SEQ // 128  # 16 key blocks
NQC = SEQ // 512  # 4 query chunks of 512
NSB = SEQ // 128  # 16 seq blocks (v / proj)

BF16 = mybir.dt.bfloat16
F32 = mybir.dt.float32
NPBF16 = ml_dtypes.bfloat16
NPFP8 = ml_dtypes.float8_e4m3
FP8 = mybir.dt.float8e4

LAST_RESULT = None  # BassKernelResults of the most recent run (for test.py)
_PROG_CACHE = {}
PROFILE = False  # test.py sets True to capture an NTFF profile / HW exec time


def _enable_profiling() -> bool:
    """Install the axon NTFF profile hook glue if the image lacks
    ``antenv.axon_hooks`` (boot degrades silently without it), and skip
    the artifact upload (no bucket access in this container)."""
    try:
        from antenv.axon_hooks import get_axon_ntff_profile_hook  # noqa: F401

        ok = get_axon_ntff_profile_hook() is not None
    except ImportError:
        ok = False
        import antenv

        mod = types.ModuleType("antenv.axon_hooks")
        mod._hook = None
        mod.set_axon_ntff_profile_hook = lambda h: setattr(mod, "_hook", h)
        mod.get_axon_ntff_profile_hook = lambda: mod._hook
        sys.modules["antenv.axon_hooks"] = mod
        antenv.axon_hooks = mod
        try:
            from trn_agent_boot.trn_boot import _ntff_profile_via_ctypes

            hook = _ntff_profile_via_ctypes("/opt/axon/libaxon_pjrt.so")
            if hook is not None:
                mod._hook = hook
                ok = True
        except Exception as e:  # profiling is best-effort
            print(f"ntff profile hook install failed: {e}", file=sys.stderr)
    if ok:
        import concourse.bass_utils as _bu

        _bu.upload_artifacts = lambda tmpdir: tmpdir
    return ok


def _build_program(causal: bool, exp_scale: float) -> bass.Bass:
    nc = bacc.Bacc(None)
    S = SEQ

    xT_d = nc.dram_tensor("xT", [D_MODEL, S], BF16, kind="ExternalInput")
    wqT_d = nc.dram_tensor("wqT", [D_MODEL, R_LOCAL], FP8, kind="ExternalInput")
    wkT_d = nc.dram_tensor("wkT", [D_MODEL, R_LOCAL], FP8, kind="ExternalInput")
    wvT_d = nc.dram_tensor("wvT", [D_MODEL, R_LOCAL], FP8, kind="ExternalInput")
    woT_d = nc.dram_tensor("woT", [R_LOCAL, D_MODEL], BF16, kind="ExternalInput")
    # cos rows 0:64, sin rows 64:128
    cs_d = nc.dram_tensor("cossinT", [128, S], BF16, kind="ExternalInput")
    # swapped: sin rows 0:64, cos rows 64:128 (keeps TensorTensor base partitions equal)
    sc_d = nc.dram_tensor("sincosT", [128, S], BF16, kind="ExternalInput")
    if causal:
        # 16 transposed diagonal 128x128 mask blocks, side by side
        maskd_d = nc.dram_tensor("maskd", [128, S], BF16, kind="ExternalInput")
    else:
        maskf_d = nc.dram_tensor("maskf", [S, S], BF16, kind="ExternalInput")
    out_d = nc.dram_tensor("out", [S, D_MODEL], BF16, kind="ExternalOutput")

    xT_v = xT_d[:].rearrange("(mo p) s -> p mo s", p=128)
    wqT_v = wqT_d[:].rearrange("(mo p) r -> p mo r", p=128)
    wkT_v = wkT_d[:].rearrange("(mo p) r -> p mo r", p=128)
    wvT_v = wvT_d[:].rearrange("(mo p) r -> p mo r", p=128)
    woT_v = woT_d[:].rearrange("(h p) o -> p h o", p=128)
    if not causal:
        maskf_v = maskf_d[:].rearrange("(ko p) q -> p ko q", p=128)

    with tile.TileContext(nc) as tc:
        with tc.tile_pool(name="pers", bufs=1) as pers:
            # ---- persistent SBUF tensors (live across both phases) ----
            q_rot = pers.tile([128, HPC, S], BF16, tag="qrot")
            k_rot = pers.tile([128, HPC, S], BF16, tag="krot")
            v_sb = pers.tile([128, NKI, R_LOCAL], BF16, tag="vsb")
            aoT = pers.tile([128, HPC, S], BF16, tag="aoT")
            ones_t = pers.tile([128, 128], BF16, tag="ones")
            warm = pers.tile([128, 1], BF16, tag="warm")
            if causal:
                maskd = pers.tile([128, S], BF16, tag="maskd")
            nc.vector.memset(ones_t[:, :], 1.0)
            # load the exp table set first so no ACT table switch happens
            # mid-kernel (Copy lives in every set).
            nc.scalar.activation(
                warm[:, :], ones_t[:, 0:1], mybir.ActivationFunctionType.Exp
            )

            # ================= phase A: QKV projection + RoPE =================
            with (
                tc.tile_pool(name="xtp", bufs=1) as xtp,
                tc.tile_pool(name="wp", bufs=1) as wp,
                tc.tile_pool(name="raw", bufs=2) as rawp,
                tc.tile_pool(name="w8", bufs=6) as w8p,
                tc.tile_pool(name="tmp", bufs=2) as tmpp,
                tc.tile_pool(name="psA", bufs=2, space="PSUM") as psA,
            ):
                xt = xtp.tile([128, MO, S], BF16, tag="xt")
                wq = wp.tile([128, MO, R_LOCAL], BF16, tag="wq")
                wk = wp.tile([128, MO, R_LOCAL], BF16, tag="wk")
                wv = wp.tile([128, MO, R_LOCAL], BF16, tag="wv")
                cs_t = wp.tile([128, S], BF16, tag="cs")
                sc_t = wp.tile([128, S], BF16, tag="sc")

                def load_w8(dst, view, mo):
                    st = w8p.tile([128, R_LOCAL], FP8, tag="w8")
                    nc.sync.dma_start(out=st[:, :], in_=view[:, mo, :])
                    nc.vector.tensor_copy(dst[:, mo, :], st[:, :])

                for mo in range(MO):
                    load_w8(wq, wqT_v, mo)
                    load_w8(wk, wkT_v, mo)
                    if mo < 4:
                        nc.sync.dma_start(
                            out=xt[:, mo, 0:1024], in_=xT_v[:, mo, 0:1024]
                        )
                        nc.sync.dma_start(
                            out=xt[:, mo, 1024:2048], in_=xT_v[:, mo, 1024:2048]
                        )
                    else:
                        nc.sync.dma_start(out=xt[:, mo, :], in_=xT_v[:, mo, :])
                nc.sync.dma_start(out=cs_t[:, :], in_=cs_d[:, :])
                nc.sync.dma_start(out=sc_t[:, :], in_=sc_d[:, :])
                if causal:
                    nc.sync.dma_start(out=maskd[:, :], in_=maskd_d[:, :])
                for mo in range(MO):
                    load_w8(wv, wvT_v, mo)

                def rope(dst, raw):
                    """NeoX rotary: rows 0:64 = t*c - b*s ; rows 64:128 = t*s + b*c."""
                    ta = tmpp.tile([64, S], BF16, tag="tmp")
                    tb = tmpp.tile([64, S], BF16, tag="tmp")
                    nc.vector.tensor_mul(ta[:, :], raw[0:64, :], cs_t[0:64, :])
                    nc.vector.tensor_mul(tb[:, :], raw[64:128, :], cs_t[64:128, :])
                    nc.vector.tensor_sub(dst[0:64, :], ta[:, :], tb[:, :])
                    tc2 = tmpp.tile([64, S], BF16, tag="tmp")
                    td = tmpp.tile([64, S], BF16, tag="tmp")
                    nc.vector.tensor_mul(tc2[:, :], raw[0:64, :], sc_t[0:64, :])
                    nc.vector.tensor_mul(td[:, :], raw[64:128, :], sc_t[64:128, :])
                    nc.vector.tensor_add(dst[64:128, :], tc2[:, :], td[:, :])

                # head 0 q/k with the m-loop OUTER so the matmuls consume
                # xt m-blocks as the DMAs land (startup overlap).
                qp0 = psA.tile([128, S], F32, tag="psA")
                kp0 = psA.tile([128, S], F32, tag="psA")
                for m in range(MO):
                    for c4 in range(4):
                        nc.tensor.matmul(
                            qp0[:, c4 * 512 : (c4 + 1) * 512],
                            wq[:, m, 0:128],
                            xt[:, m, c4 * 512 : (c4 + 1) * 512],
                            start=(m == 0),
                            stop=(m == MO - 1),
                        )
                        nc.tensor.matmul(
                            kp0[:, c4 * 512 : (c4 + 1) * 512],
                            wk[:, m, 0:128],
                            xt[:, m, c4 * 512 : (c4 + 1) * 512],
                            start=(m == 0),
                            stop=(m == MO - 1),
                        )
                q_raw = rawp.tile([128, S], BF16, tag="raw")
                nc.scalar.copy(q_raw[:, :], qp0[:, :])
                rope(q_rot[:, 0, :], q_raw)
                k_raw = rawp.tile([128, S], BF16, tag="raw")
                nc.scalar.copy(k_raw[:, :], kp0[:, :])
                rope(k_rot[:, 0, :], k_raw)

                def project(dst_raw, w_sb, h):
                    """q/k head projection -> bf16 raw [128, S] (xt resident)."""
                    ps = psA.tile([128, S], F32, tag="psA")
                    for c4 in range(4):
                        for m in range(MO):
                            nc.tensor.matmul(
                                ps[:, c4 * 512 : (c4 + 1) * 512],
                                w_sb[:, m, h * 128 : (h + 1) * 128],
                                xt[:, m, c4 * 512 : (c4 + 1) * 512],
                                start=(m == 0),
                                stop=(m == MO - 1),
                            )
                    nc.scalar.copy(dst_raw[:, :], ps[:, :])

                for h in range(1, HPC):
                    q_raw = rawp.tile([128, S], BF16, tag="raw")
                    project(q_raw, wq, h)
                    rope(q_rot[:, h, :], q_raw)
                    k_raw = rawp.tile([128, S], BF16, tag="raw")
                    project(k_raw, wk, h)
                    rope(k_rot[:, h, :], k_raw)

                # V projection (natural layout [s, r]); 4 seq blocks per psum
                for sb4 in range(NSB // 4):
                    ps = psA.tile([128, S], F32, tag="psA")
                    for part in range(4):
                        sb = sb4 * 4 + part
                        for m in range(MO):
                            nc.tensor.matmul(
                                ps[:, part * 512 : part * 512 + 512],
                                xt[:, m, sb * 128 : (sb + 1) * 128],
                                wv[:, m, :],
                                start=(m == 0),
                                stop=(m == MO - 1),
                            )
                    nc.scalar.copy(v_sb[:, sb4 * 4 : sb4 * 4 + 4, :], ps[:, :])

            # ================= phase B: attention + out-projection =============
            with (
                tc.tile_pool(name="wop", bufs=1) as wop,
                tc.tile_pool(name="pp", bufs=8) as ppp,
                tc.tile_pool(name="rcp", bufs=3) as rcp,
                tc.tile_pool(name="osb", bufs=4) as osbp,
                tc.tile_pool(name="mblk", bufs=4) as mblkp,
                tc.tile_pool(name="sp", bufs=4, space="PSUM") as spp,
                tc.tile_pool(name="acc", bufs=2, space="PSUM") as accp,
            ):
                wo = wop.tile([128, HPC, D_MODEL], BF16, tag="wo")
                for oc in range(D_MODEL // 512):
                    nc.sync.dma_start(
                        out=wo[:, :, oc * 512 : (oc + 1) * 512],
                        in_=woT_v[:, :, oc * 512 : (oc + 1) * 512],
                    )

                evict_flip = [0]

                for qc in range(NQC):
                    q_lo = qc * 512
                    nki_here = (4 * qc + 4) if causal else NKI
                    for h in range(HPC):
                        sav = accp.tile([128, 1024], F32, tag="acc")
                        sums = sav[:, 0:512]
                        avp = sav[:, 512:1024]
                        for ki in range(nki_here):
                            diag = causal and ki >= 4 * qc
                            q0 = 128 * (ki - 4 * qc) if diag else 0
                            spb = spp.tile([128, 512], F32, tag="sp")
                            pp = ppp.tile([128, 512], BF16, tag="pp")
                            nc.tensor.matmul(
                                spb[:, q0:512],
                                k_rot[:, h, ki * 128 : (ki + 1) * 128],
                                q_rot[:, h, q_lo + q0 : q_lo + 512],
                                start=True,
                                stop=True,
                            )
                            nc.scalar.activation(
                                pp[:, q0:512],
                                spb[:, q0:512],
                                mybir.ActivationFunctionType.Exp,
                                scale=float(exp_scale),
                            )
                            if causal:
                                if diag:
                                    nc.vector.tensor_mul(
                                        pp[:, q0 : q0 + 128],
                                        pp[:, q0 : q0 + 128],
                                        maskd[:, ki * 128 : (ki + 1) * 128],
                                    )
                            else:
                                mb = mblkp.tile([128, 512], BF16, tag="mblk")
                                nc.sync.dma_start(
                                    out=mb[:, :],
                                    in_=maskf_v[:, ki, q_lo : q_lo + 512],
                                )
                                nc.vector.tensor_mul(
                                    pp[:, 0:512], pp[:, 0:512], mb[:, :]
                                )
                            nc.tensor.matmul(
                                sums[:, q0:512],
                                ones_t[:, :],
                                pp[:, q0:512],
                                start=(ki == 0),
                                stop=(ki == nki_here - 1),
                            )
                            nc.tensor.matmul(
                                avp[:, q0:512],
                                v_sb[:, ki, h * 128 : (h + 1) * 128],
                                pp[:, q0:512],
                                start=(ki == 0),
                                stop=(ki == nki_here - 1),
                            )
                        rc = rcp.tile([128, 512], F32, tag="rc")
                        nc.vector.reciprocal_approx_fast(rc[:, :], sums[:, :])
                        nc.vector.tensor_mul(
                            aoT[:, h, q_lo : q_lo + 512], avp[:, :], rc[:, :]
                        )

                    # out-projection for this query chunk (4 seq blocks);
                    # h outer over oc pairs so each aoT stationary load
                    # serves two matmuls
                    for sb in range(4 * qc, 4 * qc + 4):
                        for oc2 in range(2):
                            op2 = accp.tile([128, 1024], F32, tag="acc")
                            for h in range(HPC):
                                lhsT = aoT[:, h, sb * 128 : (sb + 1) * 128]
                                nc.tensor.matmul(
                                    op2[:, 0:512],
                                    lhsT,
                                    wo[:, h, (2 * oc2) * 512 : (2 * oc2 + 1) * 512],
                                    start=(h == 0),
                                    stop=(h == HPC - 1),
                                )
                                nc.tensor.matmul(
                                    op2[:, 512:1024],
                                    lhsT,
                                    wo[:, h, (2 * oc2 + 1) * 512 : (2 * oc2 + 2) * 512],
                                    start=(h == 0),
                                    stop=(h == HPC - 1),
                                )
                            ob = osbp.tile([128, 1024], BF16, tag="osb")
                            if evict_flip[0] % 2 == 0:
                                nc.scalar.copy(ob[:, :], op2[:, :])
                            else:
                                nc.vector.tensor_copy(ob[:, :], op2[:, :])
                            evict_flip[0] += 1
                            nc.sync.dma_start(
                                out=out_d[
                                    sb * 128 : (sb + 1) * 128,
                                    oc2 * 1024 : (oc2 + 1) * 1024,
                                ],
                                in_=ob[:, :],
                            )

    nc.finalize()
    return nc


def _bit_quantize_ternary(w: np.ndarray):
    """Returns (ternary {-1,0,1} float32 matrix, scale) matching the reference."""
    scale = np.maximum(np.mean(np.abs(w.astype(np.float32))), np.float32(1e-5))
    t = np.clip(np.round(w.astype(np.float32) / scale), -1.0, 1.0).astype(np.float32)
    return t, float(scale)


def _host_tables():
    """cos/sin stacked [128, S]: rows 0:64 cos, rows 64:128 sin."""
    inv_freq = 1.0 / (ROPE_BASE ** (np.arange(0, D_HEAD, 2, dtype=np.float32) / D_HEAD))
    pos = np.arange(SEQ, dtype=np.float32)
    ang = pos[:, None] * inv_freq[None, :]  # [S, 64]
    cs = np.empty((128, SEQ), dtype=NPBF16)
    cs[0:64] = np.ascontiguousarray(np.cos(ang).T).astype(NPBF16)
    cs[64:128] = np.ascontiguousarray(np.sin(ang).T).astype(NPBF16)
    sc = np.empty((128, SEQ), dtype=NPBF16)
    sc[0:64] = cs[64:128]
    sc[64:128] = cs[0:64]
    return cs, sc


def kernel(x, w_qkv, w_out, mask):
    global LAST_RESULT
    x = np.asarray(x, dtype=np.float32)
    w_qkv = np.asarray(w_qkv, dtype=np.float32)
    w_out = np.asarray(w_out, dtype=np.float32)
    mask = np.asarray(mask)

    tq, sq = _bit_quantize_ternary(w_qkv)
    to, so = _bit_quantize_ternary(w_out)
    exp_scale = (sq * sq) / float(np.sqrt(D_HEAD))
    c2 = np.float32(sq * so)

    m2 = (mask.reshape(SEQ, SEQ) != 0).astype(np.float32)
    causal = bool(np.array_equal(m2, np.tril(np.ones((SEQ, SEQ), np.float32))))

    cs, sc = _host_tables()
    if causal:
        maskd = np.empty((128, SEQ), dtype=NPBF16)
        for ki in range(NKI):
            blk = m2[ki * 128 : (ki + 1) * 128, ki * 128 : (ki + 1) * 128]  # [q, k]
            maskd[:, ki * 128 : (ki + 1) * 128] = np.ascontiguousarray(blk.T).astype(
                NPBF16
            )
    else:
        maskf = np.ascontiguousarray(m2.T).astype(NPBF16)  # [kk, qq]

    key = (causal, float(exp_scale))
    if key not in _PROG_CACHE:
        _PROG_CACHE[key] = _build_program(causal, float(exp_scale))
    nc = _PROG_CACHE[key]

    in_maps = []
    for c in range(N_CORES):
        b, g = divmod(c, 4)
        rows = slice(R_LOCAL * g, R_LOCAL * (g + 1))
        im = {
            "xT": np.ascontiguousarray(x[b].T).astype(NPBF16),
            "wqT": np.ascontiguousarray(tq[0 * D_MODEL :][rows].T).astype(NPFP8),
            "wkT": np.ascontiguousarray(tq[1 * D_MODEL :][rows].T).astype(NPFP8),
            "wvT": np.ascontiguousarray(tq[2 * D_MODEL :][rows].T).astype(NPFP8),
            "woT": np.ascontiguousarray(to[:, rows].T).astype(NPBF16),
            "cossinT": cs,
            "sincosT": sc,
        }
        if causal:
            im["maskd"] = maskd
        else:
            im["maskf"] = maskf
        in_maps.append(im)

    do_trace = bool(PROFILE) and _enable_profiling()
    res = run_bass_kernel_spmd(nc, in_maps, list(range(N_CORES)), trace=do_trace)
    LAST_RESULT = res

    parts = [np.asarray(res.results[c]["out"]).astype(np.float32) for c in range(N_CORES)]
    out = np.stack(
        [
            parts[0] + parts[1] + parts[2] + parts[3],
            parts[4] + parts[5] + parts[6] + parts[7],
        ]
    )
    return (out * c2).astype(np.float32)



# revision 3
# speedup vs baseline: 1.2803x; 1.2803x over previous
"""Trainium2 Bass kernel for BitNet multi-head attention (nn_MultiHeadAttention_62294205661880).

Sharding: 8 cores = 2 batches x 4 head-groups (4 heads each).  Each core
computes qkv projection, RoPE, causal attention and a column-parallel slice
of the output projection for its (batch, head-group); the host sums the 4
partial out-projections per batch.

v2: fp8 DoubleRow everywhere the numerics allow.  fp8 quantization error
concentrates in the first ~512 queries (few keys -> no averaging), so query
chunk 0 takes the baseline bf16 path while chunks 1-3 run with fp8 DoubleRow
matmuls (2 contraction blocks per PE pass):
  - Q/K projection: output cols 0:512 from bf16 x, cols 512:2048 DoubleRow
    from fp8 x.
  - V projection: key blocks 0:3 from bf16 x (stored bf16+fp8), blocks 4:15
    DoubleRow (stored fp8).  wv is pre-scaled by 1/4 so v fits fp8 range
    (max |v| = 240.5 > 240 = trn-e4m3 max).
  - attention (qc>=1): softmax numerator exp() written as fp8 ki-pairs;
    denominator (ones-matmul) and AV both DoubleRow over ki-pairs.
  - out-projection (qc>=1): attention output stored fp8, head-pair DoubleRow.
Weights are ternary {-1,0,+1} (x0.25 for wv) -- exact in fp8, so DoubleRow
matmuls only add the activation-side quantization noise.

Device layout trick (unchanged from v1): everything is computed transposed.
Q_T/K_T come out of the projection as [dh, S]; scores are s_T[k, q]; the
softmax denominator sums over the partition (key) dim via an all-ones
stationary matmul; AV produces out_T[dh, q] which feeds the output projection
directly.  No on-device transposes.  Softmax skips the max-subtraction:
scores are bounded (~+-2) because the BitNet weight scale is tiny.
"""

import sys
import types

import numpy as np
import ml_dtypes

import concourse.bass as bass
import concourse.mybir as mybir
import concourse.tile as tile
from concourse import bacc
from concourse.bass_utils import run_bass_kernel_spmd

D_MODEL = 2048
N_HEADS = 16
D_HEAD = 128
SEQ = 2048
BATCH = 2
ROPE_BASE = 10000.0

N_CORES = 8
HPC = 4  # heads per core
R_LOCAL = HPC * D_HEAD  # 512 local q (or k, or v) rows per core
MO = D_MODEL // 128  # 16 contraction blocks
MO2 = MO // 2  # 8 contraction block pairs
NKI = SEQ // 128  # 16 key blocks
NQC = SEQ // 512  # 4 query chunks of 512
NSB = SEQ // 128  # 16 seq blocks (v / proj)
KI_CLEAN = 4  # key blocks with a bf16 copy (for the clean qc=0 path)

BF16 = mybir.dt.bfloat16
F32 = mybir.dt.float32
NPBF16 = ml_dtypes.bfloat16
NPFP8 = ml_dtypes.float8_e4m3
FP8 = mybir.dt.float8e4
DR = mybir.MatmulPerfMode.DoubleRow

LAST_RESULT = None  # BassKernelResults of the most recent run (for test.py)
_PROG_CACHE = {}
PROFILE = False  # test.py sets True to capture an NTFF profile / HW exec time


def _enable_profiling() -> bool:
    """Install the axon NTFF profile hook glue if the image lacks
    ``antenv.axon_hooks`` (boot degrades silently without it), and skip
    the artifact upload (no bucket access in this container)."""
    try:
        from antenv.axon_hooks import get_axon_ntff_profile_hook  # noqa: F401

        ok = get_axon_ntff_profile_hook() is not None
    except ImportError:
        ok = False
        import antenv

        mod = types.ModuleType("antenv.axon_hooks")
        mod._hook = None
        mod.set_axon_ntff_profile_hook = lambda h: setattr(mod, "_hook", h)
        mod.get_axon_ntff_profile_hook = lambda: mod._hook
        sys.modules["antenv.axon_hooks"] = mod
        antenv.axon_hooks = mod
        try:
            from trn_agent_boot.trn_boot import _ntff_profile_via_ctypes

            hook = _ntff_profile_via_ctypes("/opt/axon/libaxon_pjrt.so")
            if hook is not None:
                mod._hook = hook
                ok = True
        except Exception as e:  # profiling is best-effort
            print(f"ntff profile hook install failed: {e}", file=sys.stderr)
    if ok:
        import concourse.bass_utils as _bu

        _bu.upload_artifacts = lambda tmpdir: tmpdir
    return ok


def _build_program(exp_scale: float) -> bass.Bass:
    nc = bacc.Bacc(None)
    S = SEQ

    xT8_d = nc.dram_tensor("xT8", [D_MODEL, S], FP8, kind="ExternalInput")
    xT16_d = nc.dram_tensor("xT16", [D_MODEL, 512], BF16, kind="ExternalInput")
    wqT_d = nc.dram_tensor("wqT", [D_MODEL, R_LOCAL], FP8, kind="ExternalInput")
    wkT_d = nc.dram_tensor("wkT", [D_MODEL, R_LOCAL], FP8, kind="ExternalInput")
    wvT_d = nc.dram_tensor("wvT", [D_MODEL, R_LOCAL], FP8, kind="ExternalInput")
    woT_d = nc.dram_tensor("woT", [R_LOCAL, D_MODEL], FP8, kind="ExternalInput")
    # cos rows 0:64, sin rows 64:128
    cs_d = nc.dram_tensor("cossinT", [128, S], BF16, kind="ExternalInput")
    # swapped: sin rows 0:64, cos rows 64:128 (keeps TensorTensor base partitions equal)
    sc_d = nc.dram_tensor("sincosT", [128, S], BF16, kind="ExternalInput")
    # 16 transposed diagonal 128x128 mask blocks, side by side
    maskd16_d = nc.dram_tensor("maskd16", [128, S], BF16, kind="ExternalInput")
    maskd8_d = nc.dram_tensor("maskd8", [128, S], FP8, kind="ExternalInput")
    out_d = nc.dram_tensor("out", [S, D_MODEL], BF16, kind="ExternalOutput")

    xT8_v = xT8_d[:].rearrange("(mo p) s -> p mo s", p=128)
    xT16_v = xT16_d[:].rearrange("(mo p) s -> p mo s", p=128)
    wqT_v = wqT_d[:].rearrange("(mo p) r -> p mo r", p=128)
    wkT_v = wkT_d[:].rearrange("(mo p) r -> p mo r", p=128)
    wvT_v = wvT_d[:].rearrange("(mo p) r -> p mo r", p=128)
    woT_v = woT_d[:].rearrange("(h p) o -> p h o", p=128)

    with tile.TileContext(nc) as tc:
        with tc.tile_pool(name="pers", bufs=1) as pers:
            # ---- persistent SBUF tensors (live across both phases) ----
            q_rot = pers.tile([128, HPC, S], BF16, tag="qrot")
            k_rot = pers.tile([128, HPC, S], BF16, tag="krot")
            v8 = pers.tile([128, NKI, R_LOCAL], FP8, tag="v8")
            v16 = pers.tile([128, KI_CLEAN, R_LOCAL], BF16, tag="v16")
            aoT8 = pers.tile([128, HPC, S], FP8, tag="aoT8")
            aoT16 = pers.tile([128, HPC, 512], BF16, tag="aoT16")
            ones16 = pers.tile([128, 128], BF16, tag="ones16")
            ones8 = pers.tile([128, 2, 128], FP8, tag="ones8")
            warm = pers.tile([128, 1], BF16, tag="warm")
            maskd16 = pers.tile([128, S], BF16, tag="maskd16")
            maskd8 = pers.tile([128, S], FP8, tag="maskd8")
            nc.vector.memset(ones16[:, :], 1.0)
            nc.vector.memset(ones8[:, :, :], 1.0)
            # load the exp table set first so no ACT table switch happens
            # mid-kernel (Copy lives in every set).
            nc.scalar.activation(
                warm[:, :], ones16[:, 0:1], mybir.ActivationFunctionType.Exp
            )

            # ================= phase A: QKV projection + RoPE =================
            with (
                tc.tile_pool(name="xtp", bufs=1) as xtp,
                tc.tile_pool(name="wp", bufs=1) as wp,
                tc.tile_pool(name="raw", bufs=2) as rawp,
                tc.tile_pool(name="tmp", bufs=2) as tmpp,
                tc.tile_pool(name="psA", bufs=2, space="PSUM") as psA,
            ):
                xt8 = xtp.tile([128, MO, S], FP8, tag="xt8")
                xt16 = xtp.tile([128, MO, 512], BF16, tag="xt16")
                wq = wp.tile([128, MO, R_LOCAL], FP8, tag="wq")
                wk = wp.tile([128, MO, R_LOCAL], FP8, tag="wk")
                wv = wp.tile([128, MO, R_LOCAL], FP8, tag="wv")
                cs_t = wp.tile([128, S], BF16, tag="cs")
                sc_t = wp.tile([128, S], BF16, tag="sc")

                # startup-ordered DMAs: head-0 DR matmuls consume (wq, wk, xt8)
                # m-pairs as they land.
                for m2 in range(MO2):
                    sl = slice(2 * m2, 2 * m2 + 2)
                    nc.sync.dma_start(out=wq[:, sl, :], in_=wqT_v[:, sl, :])
                    nc.sync.dma_start(out=wk[:, sl, :], in_=wkT_v[:, sl, :])
                    nc.sync.dma_start(out=xt8[:, sl, :], in_=xT8_v[:, sl, :])
                for m4 in range(4):
                    sl = slice(4 * m4, 4 * m4 + 4)
                    nc.sync.dma_start(out=xt16[:, sl, :], in_=xT16_v[:, sl, :])
                nc.sync.dma_start(out=cs_t[:, :], in_=cs_d[:, :])
                nc.sync.dma_start(out=sc_t[:, :], in_=sc_d[:, :])
                for m2 in range(MO2):
                    sl = slice(2 * m2, 2 * m2 + 2)
                    nc.sync.dma_start(out=wv[:, sl, :], in_=wvT_v[:, sl, :])
                nc.sync.dma_start(out=maskd16[:, :], in_=maskd16_d[:, :])
                nc.sync.dma_start(out=maskd8[:, :], in_=maskd8_d[:, :])

                def rope(dst, raw):
                    """NeoX rotary: rows 0:64 = t*c - b*s ; rows 64:128 = t*s + b*c."""
                    ta = tmpp.tile([64, S], BF16, tag="tmp")
                    tb = tmpp.tile([64, S], BF16, tag="tmp")
                    nc.vector.tensor_mul(ta[:, :], raw[0:64, :], cs_t[0:64, :])
                    nc.vector.tensor_mul(tb[:, :], raw[64:128, :], cs_t[64:128, :])
                    nc.vector.tensor_sub(dst[0:64, :], ta[:, :], tb[:, :])
                    tc2 = tmpp.tile([64, S], BF16, tag="tmp")
                    td = tmpp.tile([64, S], BF16, tag="tmp")
                    nc.vector.tensor_mul(tc2[:, :], raw[0:64, :], sc_t[0:64, :])
                    nc.vector.tensor_mul(td[:, :], raw[64:128, :], sc_t[64:128, :])
                    nc.vector.tensor_add(dst[64:128, :], tc2[:, :], td[:, :])

                # head 0 q/k with the m-loop OUTER so the DR matmuls consume
                # xt8 m-pairs as the DMAs land (startup overlap).
                qp0 = psA.tile([128, S], F32, tag="psA")
                kp0 = psA.tile([128, S], F32, tag="psA")
                for m2 in range(MO2):
                    msl = slice(2 * m2, 2 * m2 + 2)
                    for c4 in range(1, 4):
                        csl = slice(c4 * 512, (c4 + 1) * 512)
                        nc.tensor.matmul(
                            qp0[:, csl],
                            wq[:, msl, 0:128],
                            xt8[:, msl, csl],
                            start=(m2 == 0),
                            stop=(m2 == MO2 - 1),
                            perf_mode=DR,
                        )
                        nc.tensor.matmul(
                            kp0[:, csl],
                            wk[:, msl, 0:128],
                            xt8[:, msl, csl],
                            start=(m2 == 0),
                            stop=(m2 == MO2 - 1),
                            perf_mode=DR,
                        )
                for m in range(MO):
                    nc.tensor.matmul(
                        qp0[:, 0:512],
                        wq[:, m, 0:128],
                        xt16[:, m, :],
                        start=(m == 0),
                        stop=(m == MO - 1),
                    )
                    nc.tensor.matmul(
                        kp0[:, 0:512],
                        wk[:, m, 0:128],
                        xt16[:, m, :],
                        start=(m == 0),
                        stop=(m == MO - 1),
                    )
                q_raw = rawp.tile([128, S], BF16, tag="raw")
                nc.scalar.copy(q_raw[:, :], qp0[:, :])
                rope(q_rot[:, 0, :], q_raw)
                k_raw = rawp.tile([128, S], BF16, tag="raw")
                nc.scalar.copy(k_raw[:, :], kp0[:, :])
                rope(k_rot[:, 0, :], k_raw)

                def project(dst_raw, w_sb, h):
                    """q/k head projection -> bf16 raw [128, S] (xt resident)."""
                    hsl = slice(h * 128, (h + 1) * 128)
                    ps = psA.tile([128, S], F32, tag="psA")
                    for m in range(MO):
                        nc.tensor.matmul(
                            ps[:, 0:512],
                            w_sb[:, m, hsl],
                            xt16[:, m, :],
                            start=(m == 0),
                            stop=(m == MO - 1),
                        )
                    for c4 in range(1, 4):
                        csl = slice(c4 * 512, (c4 + 1) * 512)
                        for m2 in range(MO2):
                            msl = slice(2 * m2, 2 * m2 + 2)
                            nc.tensor.matmul(
                                ps[:, csl],
                                w_sb[:, msl, hsl],
                                xt8[:, msl, csl],
                                start=(m2 == 0),
                                stop=(m2 == MO2 - 1),
                                perf_mode=DR,
                            )
                    nc.scalar.copy(dst_raw[:, :], ps[:, :])

                for h in range(1, HPC):
                    q_raw = rawp.tile([128, S], BF16, tag="raw")
                    project(q_raw, wq, h)
                    rope(q_rot[:, h, :], q_raw)
                    k_raw = rawp.tile([128, S], BF16, tag="raw")
                    project(k_raw, wk, h)
                    rope(k_rot[:, h, :], k_raw)

                # V projection (natural layout [s, r]); 4 seq blocks per psum.
                # Group 0 (keys 0:512) from bf16 x, stored bf16 + fp8; rest
                # DoubleRow from fp8 x, stored fp8 only.
                for sb4 in range(NSB // 4):
                    ps = psA.tile([128, S], F32, tag="psA")
                    for part in range(4):
                        sb = sb4 * 4 + part
                        ssl = slice(sb * 128, (sb + 1) * 128)
                        psl = slice(part * 512, part * 512 + 512)
                        if sb4 == 0:
                            for m in range(MO):
                                nc.tensor.matmul(
                                    ps[:, psl],
                                    xt16[:, m, ssl],
                                    wv[:, m, :],
                                    start=(m == 0),
                                    stop=(m == MO - 1),
                                )
                        else:
                            for m2 in range(MO2):
                                msl = slice(2 * m2, 2 * m2 + 2)
                                nc.tensor.matmul(
                                    ps[:, psl],
                                    xt8[:, msl, ssl],
                                    wv[:, msl, :],
                                    start=(m2 == 0),
                                    stop=(m2 == MO2 - 1),
                                    perf_mode=DR,
                                )
                    ksl = slice(sb4 * 4, sb4 * 4 + 4)
                    if sb4 == 0:
                        nc.scalar.copy(v16[:, :, :], ps[:, :])
                        nc.vector.tensor_copy(v8[:, ksl, :], ps[:, :])
                    elif sb4 == 2:
                        nc.scalar.copy(v8[:, ksl, :], ps[:, :])
                    else:
                        nc.vector.tensor_copy(v8[:, ksl, :], ps[:, :])

            # ================= phase B: attention + out-projection =============
            with (
                tc.tile_pool(name="wop", bufs=1) as wop,
                tc.tile_pool(name="pp8", bufs=4) as pp8p,
                tc.tile_pool(name="pp16", bufs=4) as pp16p,
                tc.tile_pool(name="rcp", bufs=3) as rcp,
                tc.tile_pool(name="osb", bufs=4) as osbp,
                tc.tile_pool(name="sp", bufs=2, space="PSUM") as spp,
                tc.tile_pool(name="acc", bufs=2, space="PSUM") as accp,
            ):
                wo8 = wop.tile([128, HPC, D_MODEL], FP8, tag="wo8")
                for oc in range(D_MODEL // 512):
                    nc.sync.dma_start(
                        out=wo8[:, :, oc * 512 : (oc + 1) * 512],
                        in_=woT_v[:, :, oc * 512 : (oc + 1) * 512],
                    )

                evict_flip = [0]

                for qc in range(NQC):
                    q_lo = qc * 512
                    for h in range(HPC):
                        hsl = slice(h * 128, (h + 1) * 128)
                        sav = accp.tile([128, 1024], F32, tag="acc")
                        sums = sav[:, 0:512]
                        avp = sav[:, 512:1024]
                        if qc == 0:
                            # clean bf16 path (all 4 ki blocks are diagonal)
                            for ki in range(4):
                                q0 = 128 * ki
                                ksl = slice(ki * 128, (ki + 1) * 128)
                                spb = spp.tile([128, 1024], F32, tag="sp")
                                pp16 = pp16p.tile([128, 512], BF16, tag="pp16")
                                nc.tensor.matmul(
                                    spb[:, q0:512],
                                    k_rot[:, h, ksl],
                                    q_rot[:, h, q0:512],
                                    start=True,
                                    stop=True,
                                )
                                nc.scalar.activation(
                                    pp16[:, q0:512],
                                    spb[:, q0:512],
                                    mybir.ActivationFunctionType.Exp,
                                    scale=float(exp_scale),
                                )
                                nc.vector.tensor_mul(
                                    pp16[:, q0 : q0 + 128],
                                    pp16[:, q0 : q0 + 128],
                                    maskd16[:, ksl],
                                )
                                nc.tensor.matmul(
                                    sums[:, q0:512],
                                    ones16[:, :],
                                    pp16[:, q0:512],
                                    start=(ki == 0),
                                    stop=(ki == 3),
                                )
                                nc.tensor.matmul(
                                    avp[:, q0:512],
                                    v16[:, ki, hsl],
                                    pp16[:, q0:512],
                                    start=(ki == 0),
                                    stop=(ki == 3),
                                )
                            rc = rcp.tile([128, 512], F32, tag="rc")
                            nc.vector.reciprocal_approx_fast(rc[:, :], sums[:, :])
                            nc.vector.tensor_mul(
                                aoT16[:, h, 0:512], avp[:, :], rc[:, :]
                            )
                        else:
                            npair = 2 * qc + 2
                            for kp in range(npair):
                                ki0 = 2 * kp
                                ki1 = 2 * kp + 1
                                k0sl = slice(ki0 * 128, (ki0 + 1) * 128)
                                k1sl = slice(ki1 * 128, (ki1 + 1) * 128)
                                diag = kp >= 2 * qc
                                spb = spp.tile([128, 1024], F32, tag="sp")
                                pp = pp8p.tile([128, 2, 512], FP8, tag="pp8")
                                if not diag:
                                    nc.tensor.matmul(
                                        spb[:, 0:512],
                                        k_rot[:, h, k0sl],
                                        q_rot[:, h, q_lo : q_lo + 512],
                                        start=True,
                                        stop=True,
                                    )
                                    nc.tensor.matmul(
                                        spb[:, 512:1024],
                                        k_rot[:, h, k1sl],
                                        q_rot[:, h, q_lo : q_lo + 512],
                                        start=True,
                                        stop=True,
                                    )
                                    nc.scalar.activation(
                                        pp[:, :, :],
                                        spb[:, 0:1024],
                                        mybir.ActivationFunctionType.Exp,
                                        scale=float(exp_scale),
                                    )
                                    q0 = 0
                                else:
                                    q0 = 128 * (ki0 - 4 * qc)
                                    q0b = q0 + 128
                                    nc.tensor.matmul(
                                        spb[:, q0:512],
                                        k_rot[:, h, k0sl],
                                        q_rot[:, h, q_lo + q0 : q_lo + 512],
                                        start=True,
                                        stop=True,
                                    )
                                    nc.tensor.matmul(
                                        spb[:, 512 + q0b : 1024],
                                        k_rot[:, h, k1sl],
                                        q_rot[:, h, q_lo + q0b : q_lo + 512],
                                        start=True,
                                        stop=True,
                                    )
                                    nc.scalar.activation(
                                        pp[:, 0, q0:512],
                                        spb[:, q0:512],
                                        mybir.ActivationFunctionType.Exp,
                                        scale=float(exp_scale),
                                    )
                                    nc.scalar.activation(
                                        pp[:, 1, q0b:512],
                                        spb[:, 512 + q0b : 1024],
                                        mybir.ActivationFunctionType.Exp,
                                        scale=float(exp_scale),
                                    )
                                    nc.vector.memset(pp[:, 1, q0:q0b], 0.0)
                                    nc.vector.tensor_mul(
                                        pp[:, 0, q0 : q0 + 128],
                                        pp[:, 0, q0 : q0 + 128],
                                        maskd8[:, k0sl],
                                    )
                                    nc.vector.tensor_mul(
                                        pp[:, 1, q0b : q0b + 128],
                                        pp[:, 1, q0b : q0b + 128],
                                        maskd8[:, k1sl],
                                    )
                                nc.tensor.matmul(
                                    sums[:, q0:512],
                                    ones8[:, :, :],
                                    pp[:, :, q0:512],
                                    start=(kp == 0),
                                    stop=(kp == npair - 1),
                                    perf_mode=DR,
                                )
                                nc.tensor.matmul(
                                    avp[:, q0:512],
                                    v8[:, ki0 : ki0 + 2, hsl],
                                    pp[:, :, q0:512],
                                    start=(kp == 0),
                                    stop=(kp == npair - 1),
                                    perf_mode=DR,
                                )
                            rc = rcp.tile([128, 512], F32, tag="rc")
                            nc.vector.reciprocal_approx_fast(rc[:, :], sums[:, :])
                            nc.vector.tensor_mul(
                                aoT8[:, h, q_lo : q_lo + 512], avp[:, :], rc[:, :]
                            )

                    # out-projection for this query chunk (4 seq blocks)
                    for sb in range(4 * qc, 4 * qc + 4):
                        ssl = slice(sb * 128, (sb + 1) * 128)
                        for oc2 in range(2):
                            op2 = accp.tile([128, 1024], F32, tag="acc")
                            o0 = slice((2 * oc2) * 512, (2 * oc2 + 1) * 512)
                            o1 = slice((2 * oc2 + 1) * 512, (2 * oc2 + 2) * 512)
                            if qc == 0:
                                for h in range(HPC):
                                    lhsT = aoT16[:, h, ssl]
                                    nc.tensor.matmul(
                                        op2[:, 0:512],
                                        lhsT,
                                        wo8[:, h, o0],
                                        start=(h == 0),
                                        stop=(h == HPC - 1),
                                    )
                                    nc.tensor.matmul(
                                        op2[:, 512:1024],
                                        lhsT,
                                        wo8[:, h, o1],
                                        start=(h == 0),
                                        stop=(h == HPC - 1),
                                    )
                            else:
                                for hp in range(2):
                                    hpsl = slice(2 * hp, 2 * hp + 2)
                                    lhsT = aoT8[:, hpsl, ssl]
                                    nc.tensor.matmul(
                                        op2[:, 0:512],
                                        lhsT,
                                        wo8[:, hpsl, o0],
                                        start=(hp == 0),
                                        stop=(hp == 1),
                                        perf_mode=DR,
                                    )
                                    nc.tensor.matmul(
                                        op2[:, 512:1024],
                                        lhsT,
                                        wo8[:, hpsl, o1],
                                        start=(hp == 0),
                                        stop=(hp == 1),
                                        perf_mode=DR,
                                    )
                            ob = osbp.tile([128, 1024], BF16, tag="osb")
                            if evict_flip[0] % 2 == 0:
                                nc.scalar.copy(ob[:, :], op2[:, :])
                            else:
                                nc.vector.tensor_copy(ob[:, :], op2[:, :])
                            evict_flip[0] += 1
                            nc.sync.dma_start(
                                out=out_d[
                                    sb * 128 : (sb + 1) * 128,
                                    oc2 * 1024 : (oc2 + 1) * 1024,
                                ],
                                in_=ob[:, :],
                            )

    nc.finalize()
    return nc


def _bit_quantize_ternary(w: np.ndarray):
    """Returns (ternary {-1,0,1} float32 matrix, scale) matching the reference."""
    scale = np.maximum(np.mean(np.abs(w.astype(np.float32))), np.float32(1e-5))
    t = np.clip(np.round(w.astype(np.float32) / scale), -1.0, 1.0).astype(np.float32)
    return t, float(scale)


def _host_tables():
    """cos/sin stacked [128, S]: rows 0:64 cos, rows 64:128 sin."""
    inv_freq = 1.0 / (ROPE_BASE ** (np.arange(0, D_HEAD, 2, dtype=np.float32) / D_HEAD))
    pos = np.arange(SEQ, dtype=np.float32)
    ang = pos[:, None] * inv_freq[None, :]  # [S, 64]
    cs = np.empty((128, SEQ), dtype=NPBF16)
    cs[0:64] = np.ascontiguousarray(np.cos(ang).T).astype(NPBF16)
    cs[64:128] = np.ascontiguousarray(np.sin(ang).T).astype(NPBF16)
    sc = np.empty((128, SEQ), dtype=NPBF16)
    sc[0:64] = cs[64:128]
    sc[64:128] = cs[0:64]
    return cs, sc


def kernel(x, w_qkv, w_out, mask):
    global LAST_RESULT
    x = np.asarray(x, dtype=np.float32)
    w_qkv = np.asarray(w_qkv, dtype=np.float32)
    w_out = np.asarray(w_out, dtype=np.float32)
    mask = np.asarray(mask)

    tq, sq = _bit_quantize_ternary(w_qkv)
    to, so = _bit_quantize_ternary(w_out)
    exp_scale = (sq * sq) / float(np.sqrt(D_HEAD))
    # wv is pre-scaled by 1/4 on upload (fp8 range); compensate here.
    c2 = np.float32(sq * so * 4.0)

    m2 = (mask.reshape(SEQ, SEQ) != 0).astype(np.float32)
    causal = bool(np.array_equal(m2, np.tril(np.ones((SEQ, SEQ), np.float32))))
    assert causal, "kernel specialized for the causal mask"

    cs, sc = _host_tables()
    maskd16 = np.empty((128, SEQ), dtype=NPBF16)
    for ki in range(NKI):
        blk = m2[ki * 128 : (ki + 1) * 128, ki * 128 : (ki + 1) * 128]  # [q, k]
        maskd16[:, ki * 128 : (ki + 1) * 128] = np.ascontiguousarray(blk.T).astype(
            NPBF16
        )
    maskd8 = maskd16.astype(NPFP8)

    key = float(exp_scale)
    if key not in _PROG_CACHE:
        _PROG_CACHE[key] = _build_program(float(exp_scale))
    nc = _PROG_CACHE[key]

    in_maps = []
    for c in range(N_CORES):
        b, g = divmod(c, 4)
        rows = slice(R_LOCAL * g, R_LOCAL * (g + 1))
        xT = np.ascontiguousarray(x[b].T)
        im = {
            "xT8": xT.astype(NPFP8),
            "xT16": np.ascontiguousarray(xT[:, 0:512]).astype(NPBF16),
            "wqT": np.ascontiguousarray(tq[0 * D_MODEL :][rows].T).astype(NPFP8),
            "wkT": np.ascontiguousarray(tq[1 * D_MODEL :][rows].T).astype(NPFP8),
            "wvT": np.ascontiguousarray(tq[2 * D_MODEL :][rows].T * 0.25).astype(
                NPFP8
            ),
            "woT": np.ascontiguousarray(to[:, rows].T).astype(NPFP8),
            "cossinT": cs,
            "sincosT": sc,
            "maskd16": maskd16,
            "maskd8": maskd8,
        }
        in_maps.append(im)

    do_trace = bool(PROFILE) and _enable_profiling()
    res = run_bass_kernel_spmd(nc, in_maps, list(range(N_CORES)), trace=do_trace)
    LAST_RESULT = res

    parts = [np.asarray(res.results[c]["out"]).astype(np.float32) for c in range(N_CORES)]
    out = np.stack(
        [
            parts[0] + parts[1] + parts[2] + parts[3],
            parts[4] + parts[5] + parts[6] + parts[7],
        ]
    )
    return (out * c2).astype(np.float32)


# revision 8
# speedup vs baseline: 1.3336x; 1.0416x over previous
"""Trainium2 Bass kernel for BitNet multi-head attention (nn_MultiHeadAttention_62294205661880).

Sharding: 8 cores = 2 batches x 4 head-groups (4 heads each).  Each core
computes qkv projection, RoPE, causal attention and a column-parallel slice
of the output projection for its (batch, head-group); the host sums the 4
partial out-projections per batch.

v3: fp8 DoubleRow everywhere except the first 128 queries.  fp8 quantization
error concentrates in the earliest rows (few keys -> no averaging), so rows
0:128 take a bf16 "clean" path while everything else runs fp8 DoubleRow
matmuls (2 contraction blocks per PE pass):
  - Q/K projection: output cols 0:128 from bf16 x, cols 128:2048 DoubleRow
    from fp8 x.
  - V projection: key block 0 from bf16 x (stored bf16+fp8), blocks 1:15
    DoubleRow (stored fp8).  wv is pre-scaled by 1/4 so v fits fp8 range
    (max |v| = 240.5 > 240 = trn-e4m3 max).
  - attention: softmax numerator exp() written as fp8 ki-pairs; denominator
    (ones-matmul) and AV both DoubleRow over ki-pairs.  Rows 0:128 of query
    chunk 0 keep a bf16 sub-path (scores block ki0 -> bf16 exp/sums/AV).
  - out-projection: attention output stored fp8, head-pair DoubleRow
    (seq block 0 bf16 from the clean attention output).
Weights are ternary {-1,0,+1} (x0.25 for wv) -- exact in fp8, so DoubleRow
matmuls only add the activation-side quantization noise.  Simulated
rel-err 0.0045 vs the 2e-2 gate.

Input DMAs are split across both hardware DGE queues (sync + scalar
engines) so the projection matmuls are fed at ~2x single-queue bandwidth;
out-projection emission is software-pipelined behind the next chunk's
first score matmuls to hide the aoT dependency chain.

Device layout trick (unchanged): everything is computed transposed.  Q_T/K_T
come out of the projection as [dh, S]; scores are s_T[k, q]; the softmax
denominator sums over the partition (key) dim via an all-ones stationary
matmul; AV produces out_T[dh, q] which feeds the output projection directly.
No on-device transposes.  Softmax skips the max-subtraction: scores are
bounded (~+-2) because the BitNet weight scale is tiny.
"""

import sys
import types

import numpy as np
import ml_dtypes

import concourse.bass as bass
import concourse.mybir as mybir
import concourse.tile as tile
from concourse import bacc
from concourse.bass_utils import run_bass_kernel_spmd

D_MODEL = 2048
N_HEADS = 16
D_HEAD = 128
SEQ = 2048
BATCH = 2
ROPE_BASE = 10000.0

N_CORES = 8
HPC = 4  # heads per core
R_LOCAL = HPC * D_HEAD  # 512 local q (or k, or v) rows per core
MO = D_MODEL // 128  # 16 contraction blocks
MO2 = MO // 2  # 8 contraction block pairs
NKI = SEQ // 128  # 16 key blocks
NQC = SEQ // 512  # 4 query chunks of 512
NSB = SEQ // 128  # 16 seq blocks (v / proj)
CL = 128  # clean (bf16-path) rows

BF16 = mybir.dt.bfloat16
F32 = mybir.dt.float32
NPBF16 = ml_dtypes.bfloat16
NPFP8 = ml_dtypes.float8_e4m3
FP8 = mybir.dt.float8e4
DR = mybir.MatmulPerfMode.DoubleRow

LAST_RESULT = None  # BassKernelResults of the most recent run (for test.py)
_PROG_CACHE = {}
PROFILE = False  # test.py sets True to capture an NTFF profile / HW exec time


def _enable_profiling() -> bool:
    """Install the axon NTFF profile hook glue if the image lacks
    ``antenv.axon_hooks`` (boot degrades silently without it), and skip
    the artifact upload (no bucket access in this container)."""
    try:
        from antenv.axon_hooks import get_axon_ntff_profile_hook  # noqa: F401

        ok = get_axon_ntff_profile_hook() is not None
    except ImportError:
        ok = False
        import antenv

        mod = types.ModuleType("antenv.axon_hooks")
        mod._hook = None
        mod.set_axon_ntff_profile_hook = lambda h: setattr(mod, "_hook", h)
        mod.get_axon_ntff_profile_hook = lambda: mod._hook
        sys.modules["antenv.axon_hooks"] = mod
        antenv.axon_hooks = mod
        try:
            from trn_agent_boot.trn_boot import _ntff_profile_via_ctypes

            hook = _ntff_profile_via_ctypes("/opt/axon/libaxon_pjrt.so")
            if hook is not None:
                mod._hook = hook
                ok = True
        except Exception as e:  # profiling is best-effort
            print(f"ntff profile hook install failed: {e}", file=sys.stderr)
    if ok:
        import concourse.bass_utils as _bu

        _bu.upload_artifacts = lambda tmpdir: tmpdir
    return ok


def _build_program(exp_scale: float) -> bass.Bass:
    nc = bacc.Bacc(None)
    S = SEQ

    xT8_d = nc.dram_tensor("xT8", [D_MODEL, S], FP8, kind="ExternalInput")
    xT16_d = nc.dram_tensor("xT16", [D_MODEL, CL], BF16, kind="ExternalInput")
    wqT_d = nc.dram_tensor("wqT", [D_MODEL, R_LOCAL], FP8, kind="ExternalInput")
    wkT_d = nc.dram_tensor("wkT", [D_MODEL, R_LOCAL], FP8, kind="ExternalInput")
    wvT_d = nc.dram_tensor("wvT", [D_MODEL, R_LOCAL], FP8, kind="ExternalInput")
    woT_d = nc.dram_tensor("woT", [R_LOCAL, D_MODEL], FP8, kind="ExternalInput")
    # cos rows 0:64, sin rows 64:128
    cs_d = nc.dram_tensor("cossinT", [128, S], BF16, kind="ExternalInput")
    # swapped: sin rows 0:64, cos rows 64:128 (keeps TensorTensor base partitions equal)
    sc_d = nc.dram_tensor("sincosT", [128, S], BF16, kind="ExternalInput")
    # transposed diagonal 128x128 mask blocks, side by side (bf16: block 0 only)
    maskd16_d = nc.dram_tensor("maskd16", [128, CL], BF16, kind="ExternalInput")
    maskd8_d = nc.dram_tensor("maskd8", [128, S], FP8, kind="ExternalInput")
    out_d = nc.dram_tensor("out", [S, D_MODEL], BF16, kind="ExternalOutput")

    xT8_v = xT8_d[:].rearrange("(mo p) s -> p mo s", p=128)
    xT16_v = xT16_d[:].rearrange("(mo p) s -> p mo s", p=128)
    wqT_v = wqT_d[:].rearrange("(mo p) r -> p mo r", p=128)
    wkT_v = wkT_d[:].rearrange("(mo p) r -> p mo r", p=128)
    wvT_v = wvT_d[:].rearrange("(mo p) r -> p mo r", p=128)
    woT_v = woT_d[:].rearrange("(h p) o -> p h o", p=128)

    with tile.TileContext(nc) as tc:
        with tc.tile_pool(name="pers", bufs=1) as pers:
            # ---- persistent SBUF tensors (live across both phases) ----
            q_rot = pers.tile([128, HPC, S], BF16, tag="qrot")
            k_rot = pers.tile([128, HPC, S], BF16, tag="krot")
            v8 = pers.tile([128, NKI, R_LOCAL], FP8, tag="v8")
            v16 = pers.tile([128, 1, R_LOCAL], BF16, tag="v16")
            aoT8 = pers.tile([128, HPC, S], FP8, tag="aoT8")
            aoT16 = pers.tile([128, HPC, CL], BF16, tag="aoT16")
            wo8 = pers.tile([128, HPC, D_MODEL], FP8, tag="wo8")
            ones16 = pers.tile([128, 128], BF16, tag="ones16")
            ones8 = pers.tile([128, 2, 128], FP8, tag="ones8")
            warm = pers.tile([128, 1], BF16, tag="warm")
            maskd16 = pers.tile([128, CL], BF16, tag="maskd16")
            maskd8 = pers.tile([128, S], FP8, tag="maskd8")
            nc.vector.memset(ones16[:, :], 1.0)
            nc.vector.memset(ones8[:, :, :], 1.0)
            # load the exp table set first so no ACT table switch happens
            # mid-kernel (Copy lives in every set).
            nc.scalar.activation(
                warm[:, :], ones16[:, 0:1], mybir.ActivationFunctionType.Exp
            )

            # ================= phase A: QKV projection + RoPE =================
            with (
                tc.tile_pool(name="xtp", bufs=1) as xtp,
                tc.tile_pool(name="wp", bufs=1) as wp,
                tc.tile_pool(name="raw", bufs=3) as rawp,
                tc.tile_pool(name="tmp", bufs=2) as tmpp,
                tc.tile_pool(name="psA", bufs=2, space="PSUM") as psA,
            ):
                xt8 = xtp.tile([128, MO, S], FP8, tag="xt8")  # cols 0:128 unused
                xt16 = xtp.tile([128, MO, CL], BF16, tag="xt16")
                wq = wp.tile([128, MO, R_LOCAL], FP8, tag="wq")
                wk = wp.tile([128, MO, R_LOCAL], FP8, tag="wk")
                wv = wp.tile([128, MO, R_LOCAL], FP8, tag="wv")
                cs_t = wp.tile([128, S], BF16, tag="cs")
                sc_t = wp.tile([128, S], BF16, tag="sc")

                # Input DMAs on both hardware DGE queues (sync + scalar):
                # queue B (scalar): x fp8 pairs (first pair split so matmul 0
                # starts early), then wo8.  queue A (sync): everything else,
                # ordered by first use.  head-0 DR matmuls consume (wq, wk,
                # xt8) m-pairs as they land.
                for c4 in range(4):
                    lo = 128 if c4 == 0 else c4 * 512
                    nc.scalar.dma_start(
                        out=xt8[:, 0:2, lo : (c4 + 1) * 512],
                        in_=xT8_v[:, 0:2, lo : (c4 + 1) * 512],
                    )
                nc.sync.dma_start(out=wq[:, 0:2, :], in_=wqT_v[:, 0:2, :])
                nc.sync.dma_start(out=wk[:, 0:2, :], in_=wkT_v[:, 0:2, :])
                for m2 in range(1, MO2):
                    sl = slice(2 * m2, 2 * m2 + 2)
                    nc.scalar.dma_start(
                        out=xt8[:, sl, 128:S], in_=xT8_v[:, sl, 128:S]
                    )
                    nc.sync.dma_start(out=wq[:, sl, :], in_=wqT_v[:, sl, :])
                    nc.sync.dma_start(out=wk[:, sl, :], in_=wkT_v[:, sl, :])
                nc.sync.dma_start(out=xt16[:, :, :], in_=xT16_v[:, :, :])
                nc.sync.dma_start(out=cs_t[:, :], in_=cs_d[:, :])
                nc.sync.dma_start(out=sc_t[:, :], in_=sc_d[:, :])
                for m2 in range(MO2):
                    sl = slice(2 * m2, 2 * m2 + 2)
                    nc.sync.dma_start(out=wv[:, sl, :], in_=wvT_v[:, sl, :])
                nc.scalar.dma_start(out=wo8[:, :, :], in_=woT_v[:, :, :])
                nc.sync.dma_start(out=maskd16[:, :], in_=maskd16_d[:, :])
                nc.sync.dma_start(out=maskd8[:, :], in_=maskd8_d[:, :])

                def rope(dst, raw):
                    """NeoX rotary: rows 0:64 = t*c - b*s ; rows 64:128 = t*s + b*c."""
                    ta = tmpp.tile([64, S], BF16, tag="tmp")
                    tb = tmpp.tile([64, S], BF16, tag="tmp")
                    nc.vector.tensor_mul(ta[:, :], raw[0:64, :], cs_t[0:64, :])
                    nc.vector.tensor_mul(tb[:, :], raw[64:128, :], cs_t[64:128, :])
                    nc.vector.tensor_sub(dst[0:64, :], ta[:, :], tb[:, :])
                    tc2 = tmpp.tile([64, S], BF16, tag="tmp")
                    td = tmpp.tile([64, S], BF16, tag="tmp")
                    nc.vector.tensor_mul(tc2[:, :], raw[0:64, :], sc_t[0:64, :])
                    nc.vector.tensor_mul(td[:, :], raw[64:128, :], sc_t[64:128, :])
                    nc.vector.tensor_add(dst[64:128, :], tc2[:, :], td[:, :])

                def dr_chunks(ps, w_sb, hsl, msl, first, last):
                    """DoubleRow q/k chunk matmuls for one m-pair (cols 128:2048).

                    The c4=0 chunk shares its PSUM bank (2KB zero region) with
                    the bf16 clean cols 0:128, whose matmuls come later: only
                    the first matmul per bank may carry start=True, and the
                    bank's stop stays with its last writer (the clean loop)."""
                    for c4 in range(4):
                        lo = 128 if c4 == 0 else c4 * 512
                        csl = slice(lo, (c4 + 1) * 512)
                        nc.tensor.matmul(
                            ps[:, csl],
                            w_sb[:, msl, hsl],
                            xt8[:, msl, csl],
                            start=first,
                            stop=last and c4 != 0,
                            perf_mode=DR,
                        )

                # head 0 q/k with the m-loop OUTER so the DR matmuls consume
                # xt8 m-pairs as the DMAs land (startup overlap).
                qp0 = psA.tile([128, S], F32, tag="psA")
                kp0 = psA.tile([128, S], F32, tag="psA")
                for m2 in range(MO2):
                    msl = slice(2 * m2, 2 * m2 + 2)
                    dr_chunks(qp0, wq, slice(0, 128), msl, m2 == 0, m2 == MO2 - 1)
                    dr_chunks(kp0, wk, slice(0, 128), msl, m2 == 0, m2 == MO2 - 1)
                for m in range(MO):
                    nc.tensor.matmul(
                        qp0[:, 0:CL],
                        wq[:, m, 0:128],
                        xt16[:, m, :],
                        start=False,
                        stop=(m == MO - 1),
                    )
                    nc.tensor.matmul(
                        kp0[:, 0:CL],
                        wk[:, m, 0:128],
                        xt16[:, m, :],
                        start=False,
                        stop=(m == MO - 1),
                    )
                q_raw = rawp.tile([128, S], BF16, tag="raw")
                nc.scalar.copy(q_raw[:, :], qp0[:, :])
                rope(q_rot[:, 0, :], q_raw)
                k_raw = rawp.tile([128, S], BF16, tag="raw")
                nc.scalar.copy(k_raw[:, :], kp0[:, :])
                rope(k_rot[:, 0, :], k_raw)

                def project(dst_raw, w_sb, h):
                    """q/k head projection -> bf16 raw [128, S] (xt resident)."""
                    hsl = slice(h * 128, (h + 1) * 128)
                    ps = psA.tile([128, S], F32, tag="psA")
                    for m2 in range(MO2):
                        msl = slice(2 * m2, 2 * m2 + 2)
                        dr_chunks(ps, w_sb, hsl, msl, m2 == 0, m2 == MO2 - 1)
                    for m in range(MO):
                        nc.tensor.matmul(
                            ps[:, 0:CL],
                            w_sb[:, m, hsl],
                            xt16[:, m, :],
                            start=False,
                            stop=(m == MO - 1),
                        )
                    nc.scalar.copy(dst_raw[:, :], ps[:, :])

                for h in range(1, HPC):
                    q_raw = rawp.tile([128, S], BF16, tag="raw")
                    project(q_raw, wq, h)
                    rope(q_rot[:, h, :], q_raw)
                    k_raw = rawp.tile([128, S], BF16, tag="raw")
                    project(k_raw, wk, h)
                    rope(k_rot[:, h, :], k_raw)

                # V projection (natural layout [s, r]); 4 seq blocks per psum.
                # sb0 (keys 0:128) from bf16 x, stored bf16 + fp8; the rest
                # DoubleRow from fp8 x, stored fp8 only.
                for sb4 in range(NSB // 4):
                    ps = psA.tile([128, S], F32, tag="psA")
                    for part in range(4):
                        sb = sb4 * 4 + part
                        ssl = slice(sb * 128, (sb + 1) * 128)
                        psl = slice(part * 512, part * 512 + 512)
                        if sb == 0:
                            for m in range(MO):
                                nc.tensor.matmul(
                                    ps[:, psl],
                                    xt16[:, m, :],
                                    wv[:, m, :],
                                    start=(m == 0),
                                    stop=(m == MO - 1),
                                )
                        else:
                            for m2 in range(MO2):
                                msl = slice(2 * m2, 2 * m2 + 2)
                                nc.tensor.matmul(
                                    ps[:, psl],
                                    xt8[:, msl, ssl],
                                    wv[:, msl, :],
                                    start=(m2 == 0),
                                    stop=(m2 == MO2 - 1),
                                    perf_mode=DR,
                                )
                    k0 = sb4 * 4
                    if sb4 == 0:
                        nc.scalar.copy(v16[:, 0, :], ps[:, 0:512])
                        nc.vector.tensor_copy(v8[:, 0:2, :], ps[:, 0:1024])
                        nc.scalar.copy(v8[:, 2:4, :], ps[:, 1024:2048])
                    else:
                        nc.vector.tensor_copy(
                            v8[:, k0 : k0 + 2, :], ps[:, 0:1024]
                        )
                        nc.scalar.copy(v8[:, k0 + 2 : k0 + 4, :], ps[:, 1024:2048])

            # ================= phase B: attention + out-projection =============
            with (
                tc.tile_pool(name="pp8", bufs=4) as pp8p,
                tc.tile_pool(name="ppc", bufs=2) as ppcp,
                tc.tile_pool(name="rcp", bufs=3) as rcp,
                tc.tile_pool(name="osb", bufs=4) as osbp,
                tc.tile_pool(name="sp", bufs=2, space="PSUM") as spp,
                tc.tile_pool(name="acc", bufs=2, space="PSUM") as accp,
            ):
                EXP = mybir.ActivationFunctionType.Exp
                esc = float(exp_scale)
                evict_flip = [0]

                def scores_pair(qc, h, kp, pp):
                    """scores + exp + mask for ki-pair kp -> pp [128, 2, 512].
                    Returns q0, the first valid column of the pair."""
                    q_lo = qc * 512
                    ki0, ki1 = 2 * kp, 2 * kp + 1
                    k0sl = slice(ki0 * 128, (ki0 + 1) * 128)
                    k1sl = slice(ki1 * 128, (ki1 + 1) * 128)
                    spb = spp.tile([128, 1024], F32, tag="sp")
                    if kp < 2 * qc:  # off-diagonal pair
                        nc.tensor.matmul(
                            spb[:, 0:512],
                            k_rot[:, h, k0sl],
                            q_rot[:, h, q_lo : q_lo + 512],
                            start=True,
                            stop=True,
                        )
                        nc.tensor.matmul(
                            spb[:, 512:1024],
                            k_rot[:, h, k1sl],
                            q_rot[:, h, q_lo : q_lo + 512],
                            start=True,
                            stop=True,
                        )
                        nc.scalar.activation(pp[:, :, :], spb[:, 0:1024], EXP, scale=esc)
                        return 0
                    q0 = 128 * (ki0 - 4 * qc)
                    q0b = q0 + 128
                    nc.tensor.matmul(
                        spb[:, q0:512],
                        k_rot[:, h, k0sl],
                        q_rot[:, h, q_lo + q0 : q_lo + 512],
                        start=True,
                        stop=True,
                    )
                    nc.tensor.matmul(
                        spb[:, 512 + q0b : 1024],
                        k_rot[:, h, k1sl],
                        q_rot[:, h, q_lo + q0b : q_lo + 512],
                        start=True,
                        stop=True,
                    )
                    nc.scalar.activation(
                        pp[:, 0, q0:512], spb[:, q0:512], EXP, scale=esc
                    )
                    nc.scalar.activation(
                        pp[:, 1, q0b:512], spb[:, 512 + q0b : 1024], EXP, scale=esc
                    )
                    nc.vector.memset(pp[:, 1, q0:q0b], 0.0)
                    nc.vector.tensor_mul(
                        pp[:, 0, q0 : q0 + 128],
                        pp[:, 0, q0 : q0 + 128],
                        maskd8[:, k0sl],
                    )
                    nc.vector.tensor_mul(
                        pp[:, 1, q0b : q0b + 128],
                        pp[:, 1, q0b : q0b + 128],
                        maskd8[:, k1sl],
                    )
                    return q0

                def sums_av_pair(h, kp, npair, pp, q0, sums, avp):
                    hsl = slice(h * 128, (h + 1) * 128)
                    nc.tensor.matmul(
                        sums[:, q0:512],
                        ones8[:, :, :],
                        pp[:, :, q0:512],
                        start=(kp == 0),
                        stop=(kp == npair - 1),
                        perf_mode=DR,
                    )
                    nc.tensor.matmul(
                        avp[:, q0:512],
                        v8[:, 2 * kp : 2 * kp + 2, hsl],
                        pp[:, :, q0:512],
                        start=(kp == 0),
                        stop=(kp == npair - 1),
                        perf_mode=DR,
                    )

                def finish_head(qc, h, sums, avp):
                    rc = rcp.tile([128, 512], F32, tag="rc")
                    nc.vector.reciprocal_approx_fast(rc[:, :], sums[:, :])
                    if qc == 0:
                        nc.vector.tensor_mul(
                            aoT16[:, h, :], avp[:, 0:CL], rc[:, 0:CL]
                        )
                        nc.vector.tensor_mul(
                            aoT8[:, h, CL:512], avp[:, CL:512], rc[:, CL:512]
                        )
                    else:
                        q_lo = qc * 512
                        nc.vector.tensor_mul(
                            aoT8[:, h, q_lo : q_lo + 512], avp[:, :], rc[:, :]
                        )

                def attn_head0_clean(h):
                    """qc=0: rows 0:128 bf16 via ki0; rows 128:512 fp8 pairs."""
                    hsl = slice(h * 128, (h + 1) * 128)
                    sav = accp.tile([128, 1024], F32, tag="acc")
                    sums, avp = sav[:, 0:512], sav[:, 512:1024]
                    # pair 0 (ki0 full block, ki1 diag at q0=128)
                    spb = spp.tile([128, 1024], F32, tag="sp")
                    pp = pp8p.tile([128, 2, 512], FP8, tag="pp8")
                    ppc = ppcp.tile([128, CL], BF16, tag="ppc")
                    nc.tensor.matmul(
                        spb[:, 0:512],
                        k_rot[:, h, 0:128],
                        q_rot[:, h, 0:512],
                        start=True,
                        stop=True,
                    )
                    nc.tensor.matmul(
                        spb[:, 512 + 128 : 1024],
                        k_rot[:, h, 128:256],
                        q_rot[:, h, 128:512],
                        start=True,
                        stop=True,
                    )
                    # fp8 exps cover q 128:512; bf16 exp covers q 0:128
                    nc.scalar.activation(pp[:, 0, 128:512], spb[:, 128:512], EXP, scale=esc)
                    nc.scalar.activation(
                        pp[:, 1, 128:512], spb[:, 640:1024], EXP, scale=esc
                    )
                    nc.scalar.activation(ppc[:, :], spb[:, 0:CL], EXP, scale=esc)
                    nc.vector.tensor_mul(
                        pp[:, 1, 128:256], pp[:, 1, 128:256], maskd8[:, 128:256]
                    )
                    nc.vector.tensor_mul(ppc[:, :], ppc[:, :], maskd16[:, :])
                    nc.tensor.matmul(
                        sums[:, 128:512],
                        ones8[:, :, :],
                        pp[:, :, 128:512],
                        start=True,
                        stop=False,
                        perf_mode=DR,
                    )
                    nc.tensor.matmul(
                        avp[:, 128:512],
                        v8[:, 0:2, hsl],
                        pp[:, :, 128:512],
                        start=True,
                        stop=False,
                        perf_mode=DR,
                    )
                    # same PSUM banks as the fp8 pair above: accumulate-only
                    # (the bank's start/stop live on the fp8 pair matmuls)
                    nc.tensor.matmul(
                        sums[:, 0:CL], ones16[:, :], ppc[:, :], start=False, stop=False
                    )
                    nc.tensor.matmul(
                        avp[:, 0:CL], v16[:, 0, hsl], ppc[:, :], start=False, stop=False
                    )
                    # pair 1 (ki2/ki3 diag at q0=256/384)
                    pp = pp8p.tile([128, 2, 512], FP8, tag="pp8")
                    q0 = scores_pair(0, h, 1, pp)
                    nc.tensor.matmul(
                        sums[:, q0:512],
                        ones8[:, :, :],
                        pp[:, :, q0:512],
                        start=False,
                        stop=True,
                        perf_mode=DR,
                    )
                    nc.tensor.matmul(
                        avp[:, q0:512],
                        v8[:, 2:4, hsl],
                        pp[:, :, q0:512],
                        start=False,
                        stop=True,
                        perf_mode=DR,
                    )
                    finish_head(0, h, sums, avp)

                def attn_head(qc, h, peeled=None):
                    sav = accp.tile([128, 1024], F32, tag="acc")
                    sums, avp = sav[:, 0:512], sav[:, 512:1024]
                    npair = 2 * qc + 2
                    for kp in range(npair):
                        if peeled is not None and kp < len(peeled):
                            pp, q0 = peeled[kp]
                        else:
                            pp = pp8p.tile([128, 2, 512], FP8, tag="pp8")
                            q0 = scores_pair(qc, h, kp, pp)
                        sums_av_pair(h, kp, npair, pp, q0, sums, avp)
                    finish_head(qc, h, sums, avp)

                def outproj(qc):
                    for sb in range(4 * qc, 4 * qc + 4):
                        ssl = slice(sb * 128, (sb + 1) * 128)
                        for oc2 in range(2):
                            op2 = accp.tile([128, 1024], F32, tag="acc")
                            o0 = slice((2 * oc2) * 512, (2 * oc2 + 1) * 512)
                            o1 = slice((2 * oc2 + 1) * 512, (2 * oc2 + 2) * 512)
                            if sb == 0:
                                for h in range(HPC):
                                    lhsT = aoT16[:, h, :]
                                    nc.tensor.matmul(
                                        op2[:, 0:512],
                                        lhsT,
                                        wo8[:, h, o0],
                                        start=(h == 0),
                                        stop=(h == HPC - 1),
                                    )
                                    nc.tensor.matmul(
                                        op2[:, 512:1024],
                                        lhsT,
                                        wo8[:, h, o1],
                                        start=(h == 0),
                                        stop=(h == HPC - 1),
                                    )
                            else:
                                for hp in range(2):
                                    hpsl = slice(2 * hp, 2 * hp + 2)
                                    lhsT = aoT8[:, hpsl, ssl]
                                    nc.tensor.matmul(
                                        op2[:, 0:512],
                                        lhsT,
                                        wo8[:, hpsl, o0],
                                        start=(hp == 0),
                                        stop=(hp == 1),
                                        perf_mode=DR,
                                    )
                                    nc.tensor.matmul(
                                        op2[:, 512:1024],
                                        lhsT,
                                        wo8[:, hpsl, o1],
                                        start=(hp == 0),
                                        stop=(hp == 1),
                                        perf_mode=DR,
                                    )
                            ob = osbp.tile([128, 1024], BF16, tag="osb")
                            if evict_flip[0] % 2 == 0:
                                nc.scalar.copy(ob[:, :], op2[:, :])
                                nc.scalar.dma_start(
                                    out=out_d[ssl, oc2 * 1024 : (oc2 + 1) * 1024],
                                    in_=ob[:, :],
                                )
                            else:
                                nc.vector.tensor_copy(ob[:, :], op2[:, :])
                                nc.sync.dma_start(
                                    out=out_d[ssl, oc2 * 1024 : (oc2 + 1) * 1024],
                                    in_=ob[:, :],
                                )
                            evict_flip[0] += 1

                for h in range(HPC):
                    attn_head0_clean(h)
                for qc in range(1, NQC):
                    # peel the next chunk's first scores so the PE has work
                    # while the previous chunk's aoT dependency chain drains.
                    peeled = []
                    for kp in range(2):
                        pp = pp8p.tile([128, 2, 512], FP8, tag="pp8")
                        q0 = scores_pair(qc, 0, kp, pp)
                        peeled.append((pp, q0))
                    outproj(qc - 1)
                    attn_head(qc, 0, peeled=peeled)
                    for h in range(1, HPC):
                        attn_head(qc, h)
                outproj(NQC - 1)

    nc.finalize()
    return nc


def _bit_quantize_ternary(w: np.ndarray):
    """Returns (ternary {-1,0,1} float32 matrix, scale) matching the reference."""
    scale = np.maximum(np.mean(np.abs(w.astype(np.float32))), np.float32(1e-5))
    t = np.clip(np.round(w.astype(np.float32) / scale), -1.0, 1.0).astype(np.float32)
    return t, float(scale)


def _host_tables():
    """cos/sin stacked [128, S]: rows 0:64 cos, rows 64:128 sin."""
    inv_freq = 1.0 / (ROPE_BASE ** (np.arange(0, D_HEAD, 2, dtype=np.float32) / D_HEAD))
    pos = np.arange(SEQ, dtype=np.float32)
    ang = pos[:, None] * inv_freq[None, :]  # [S, 64]
    cs = np.empty((128, SEQ), dtype=NPBF16)
    cs[0:64] = np.ascontiguousarray(np.cos(ang).T).astype(NPBF16)
    cs[64:128] = np.ascontiguousarray(np.sin(ang).T).astype(NPBF16)
    sc = np.empty((128, SEQ), dtype=NPBF16)
    sc[0:64] = cs[64:128]
    sc[64:128] = cs[0:64]
    return cs, sc


def kernel(x, w_qkv, w_out, mask):
    global LAST_RESULT
    x = np.asarray(x, dtype=np.float32)
    w_qkv = np.asarray(w_qkv, dtype=np.float32)
    w_out = np.asarray(w_out, dtype=np.float32)
    mask = np.asarray(mask)

    tq, sq = _bit_quantize_ternary(w_qkv)
    to, so = _bit_quantize_ternary(w_out)
    exp_scale = (sq * sq) / float(np.sqrt(D_HEAD))
    # wv is pre-scaled by 1/4 on upload (fp8 range); compensate here.
    c2 = np.float32(sq * so * 4.0)

    m2 = (mask.reshape(SEQ, SEQ) != 0).astype(np.float32)
    causal = bool(np.array_equal(m2, np.tril(np.ones((SEQ, SEQ), np.float32))))
    assert causal, "kernel specialized for the causal mask"

    cs, sc = _host_tables()
    maskd8 = np.empty((128, SEQ), dtype=NPFP8)
    for ki in range(NKI):
        blk = m2[ki * 128 : (ki + 1) * 128, ki * 128 : (ki + 1) * 128]  # [q, k]
        maskd8[:, ki * 128 : (ki + 1) * 128] = np.ascontiguousarray(blk.T).astype(
            NPFP8
        )
    maskd16 = maskd8[:, 0:CL].astype(NPBF16)

    key = float(exp_scale)
    if key not in _PROG_CACHE:
        _PROG_CACHE[key] = _build_program(float(exp_scale))
    nc = _PROG_CACHE[key]

    in_maps = []
    for c in range(N_CORES):
        b, g = divmod(c, 4)
        rows = slice(R_LOCAL * g, R_LOCAL * (g + 1))
        xT = np.ascontiguousarray(x[b].T)
        im = {
            "xT8": xT.astype(NPFP8),
            "xT16": np.ascontiguousarray(xT[:, 0:CL]).astype(NPBF16),
            "wqT": np.ascontiguousarray(tq[0 * D_MODEL :][rows].T).astype(NPFP8),
            "wkT": np.ascontiguousarray(tq[1 * D_MODEL :][rows].T).astype(NPFP8),
            "wvT": np.ascontiguousarray(tq[2 * D_MODEL :][rows].T * 0.25).astype(
                NPFP8
            ),
            "woT": np.ascontiguousarray(to[:, rows].T).astype(NPFP8),
            "cossinT": cs,
            "sincosT": sc,
            "maskd16": maskd16,
            "maskd8": maskd8,
        }
        in_maps.append(im)

    do_trace = bool(PROFILE) and _enable_profiling()
    res = run_bass_kernel_spmd(nc, in_maps, list(range(N_CORES)), trace=do_trace)
    LAST_RESULT = res

    parts = [np.asarray(res.results[c]["out"]).astype(np.float32) for c in range(N_CORES)]
    out = np.stack(
        [
            parts[0] + parts[1] + parts[2] + parts[3],
            parts[4] + parts[5] + parts[6] + parts[7],
        ]
    )
    return (out * c2).astype(np.float32)
